# revision 1
# baseline (speedup 1.0000x reference)
"""CP-gate layer kernel for Trainium2 (8 NeuronCores, batch-parallel).

The reference materializes the dense 2^n x 2^n CP gate, but that matrix is
diagonal: diag entry is e^{-i*phase} on basis states where both the control
(bit 11, MSB) and target (bit 10) bits are 1, else 1.  With MSB-first
ordering those states are exactly the contiguous index range [3072, 4096).
So U @ psi is: identity on k < 3072, and a fixed complex rotation of the
tail quarter.  The batch of 64 state vectors is sharded across 8 cores
(8 states/core): each core DMA-copies the untouched 3/4 DRAM->DRAM and
rotates its tail quarter on the vector engine.

The kernel is raw manually-synced bacc (no TileContext): the whole job is
one load->rotate->store latency chain plus one independent body copy, so
Tile's scheduling/barrier machinery only adds overhead.  Every instruction
carries at most one sem wait (TRN2 limit; Bacc legalizes any extras).
Host packs the tail as a contiguous (128, 128) tile ([re | im] along the
free dim) so one DMA feeds the rotate and DVE ops run at full width.
"""

import numpy as np

N_CORES = 8
BATCH = 64
DIM = 4096
B_PER = BATCH // N_CORES          # 8 states per core
SPLIT = 3072                      # k >= SPLIT picks up the phase
TAIL = DIM - SPLIT                # 1024
NPART = 128                       # tail tile partitions: (b, km) = 8*16
HK = 64                           # tail tile cols per half: re 0:64, im 64:128
PHASE = np.pi / 4.0
C = float(np.cos(PHASE))          # cos == sin for pi/4

_cached_nc = None


def _build_nc():
    import concourse.bacc as bacc
    import concourse.bass as bass
    import concourse.mybir as mybir

    f32 = mybir.dt.float32
    i32 = mybir.dt.int32
    nc = bacc.Bacc("TRN2", target_bir_lowering=False, debug=False, num_devices=N_CORES)
    body = nc.declare_dram_parameter("body", [2, B_PER, SPLIT], f32, isOutput=False)
    tails = nc.declare_dram_parameter("tails", [NPART, 2 * HK], f32, isOutput=False)
    obody = nc.declare_dram_parameter("out_body", [2, B_PER, SPLIT], f32, isOutput=True)
    otail = nc.declare_dram_parameter("out_tail", [NPART, 2 * HK], f32, isOutput=True)

    with (
        nc.sbuf_tensor([NPART, 2 * HK], f32) as t,
        nc.sbuf_tensor([NPART, 2 * HK], f32) as s,
        nc.sbuf_tensor([NPART, 2 * HK], f32) as r,
        nc.Block() as block,
        nc.semaphore("ld") as ld,
        nc.semaphore("dve") as dve,
        nc.semaphore("cp") as cp,
        nc.semaphore("st") as st,
        nc.semaphore("prep") as prep,
    ):

        @block.sync
        def _(sp: bass.BassEngine):
            sp.dma_start(out=t[:], in_=tails[:]).then_inc(ld, 16)

        @block.gpsimd
        def _(g: bass.BassEngine):
            g.dma_start(out=obody[:, :, :], in_=body[:, :, :]).then_inc(cp, 16)
            # Zero ctx index: reuse the preamble's const-f32-0.0 [128,1] SBUF
            # tensor (all-zero bytes) bitcast to int32; the start barrier
            # already orders const init before this read.
            idx0 = nc.const_aps.aps[(f32, 0.0)].bitcast(i32)
            out4 = otail[:].rearrange("p (o n) -> p o n", o=1).unsqueeze(0)
            in4 = r[:].rearrange("p (a n) -> p a n", a=1).unsqueeze(2)
            g.kv_writeback(
                out_ap=out4, in_ap=in4, ctx_idxs_ap=idx0,
                prepare_only=True, sem=st, queue_num=0,
            ).then_inc(prep, 1)
            # Wait order matters: Bacc fuses one wait onto the trigger.
            # This order lands the critical dve wait ON the trigger ISA op
            # and leaves the early-satisfied prep wait standalone (~60 ns).
            g.wait_ge(dve, 3)
            g.wait_ge(prep, 1)
            g.trigger_dma(count=1, queue_num=0)
            g.wait_ge(cp, 16)
            g.wait_ge(st, 16)

        @block.vector
        def _(v: bass.BassEngine):
            v.wait_ge(ld, 16)
            # s_im = fl(C*im); then out_re = fl(C*re)+s_im, out_im = s_im-fl(C*re)
            # via scalar_tensor_tensor — same rounding as the reference.
            v.tensor_scalar_mul(
                s[:, HK : 2 * HK], t[:, HK : 2 * HK], C
            ).then_inc(dve, 1)
            v.wait_ge(dve, 1)
            v.scalar_tensor_tensor(
                out=r[:, 0:HK], in0=t[:, 0:HK], scalar=C, in1=s[:, HK : 2 * HK],
                op0=mybir.AluOpType.mult, op1=mybir.AluOpType.add,
            ).then_inc(dve, 1)
            v.scalar_tensor_tensor(
                out=r[:, HK : 2 * HK], in0=t[:, 0:HK], scalar=-C, in1=s[:, HK : 2 * HK],
                op0=mybir.AluOpType.mult, op1=mybir.AluOpType.add,
            ).then_inc(dve, 1)

    # Hoist the tail load into the start-barrier window: SP's barrier leg is
    # Drain (gather inc) then an EventSemaphore wait; issuing the load between
    # them starts its ~2.4us DMA pipeline at t~0 without delaying the barrier.
    # Safe: sems are zeroed by the NRT preamble before any instruction runs,
    # the load waits on nothing, and its sem inc lands long after all engines
    # left the barrier.
    SP = mybir.EngineType.SP
    fn = nc.m.functions[0]
    load_inst = None
    for b in fn.blocks:
        for i in list(b.instructions):
            if isinstance(i, mybir.InstDMACopy) and i.engine == SP:
                load_inst = i
                b.instructions.remove(i)
                break
        if load_inst is not None:
            break
    assert load_inst is not None
    main = fn.blocks[0]
    for n, i in enumerate(main.instructions):
        if isinstance(i, mybir.InstEventSemaphore) and i.engine == SP:
            main.instructions.insert(n, load_inst)
            break
    else:
        raise AssertionError("SP barrier EventSemaphore not found")

    # Hoist the body copy's dispatch into Pool's barrier window (after its
    # Drain, before its gather EventSemaphore): the ~1us SWDGE descriptor
    # gen runs during the barrier, moving the copy's completion ~250 ns
    # earlier.  Same safety argument as the load hoist.
    Pool = mybir.EngineType.Pool
    copy_inst = None
    for b in fn.blocks:
        for i in list(b.instructions):
            if isinstance(i, mybir.InstDMACopy) and i.engine == Pool:
                copy_inst = i
                b.instructions.remove(i)
                break
        if copy_inst is not None:
            break
    assert copy_inst is not None
    for n, i in enumerate(main.instructions):
        if isinstance(i, mybir.InstEventSemaphore) and i.engine == Pool:
            main.instructions.insert(n, copy_inst)
            break
    else:
        raise AssertionError("Pool barrier EventSemaphore not found")

    # Overlap the end barrier with the store: move Pool's cp/st completion
    # waits from its body into the end-barrier window (after the gather
    # phase, before Pool's release EventSemaphore).  Kernel end still gates
    # on both DMAs landing, but the barrier legs run while they are in
    # flight.
    def _wait_names(i):
        si = getattr(i, "sync_info", None)
        ow = getattr(si, "on_wait", None) or []
        return [getattr(w, "ant_name", "") for w in ow]
    moved = []
    for b in fn.blocks:
        for i in list(b.instructions):
            if i.engine == Pool and any(n in ("cp", "st") for n in _wait_names(i)):
                moved.append(i)
                b.instructions.remove(i)
    assert len(moved) == 2, [(_wait_names(i)) for i in moved]
    end_bb = fn.blocks[-1]
    release_idx = None
    for n, i in enumerate(end_bb.instructions):
        if isinstance(i, mybir.InstEventSemaphore) and i.engine == Pool:
            release_idx = n  # keep last match (release comes after gather)
    assert release_idx is not None
    end_bb.instructions[release_idx:release_idx] = moved

    nc.finalize()
    return nc


def _get_nc():
    global _cached_nc
    if _cached_nc is None:
        _cached_nc = _build_nc()
    return _cached_nc


def kernel(psi_re=None, psi_im=None, U_re=None, U_im=None, _trace=False, **_ignored):
    from concourse.bass_utils import run_bass_kernel_spmd

    psi_re = np.asarray(psi_re, dtype=np.float32).reshape(BATCH, DIM)
    psi_im = np.asarray(psi_im, dtype=np.float32).reshape(BATCH, DIM)

    nc = _get_nc()
    in_maps = []
    for i in range(N_CORES):
        re = psi_re[i * B_PER : (i + 1) * B_PER]
        im = psi_im[i * B_PER : (i + 1) * B_PER]
        body = np.ascontiguousarray(np.stack([re[:, :SPLIT], im[:, :SPLIT]]))
        tails = np.concatenate(
            [re[:, SPLIT:].reshape(NPART, HK), im[:, SPLIT:].reshape(NPART, HK)],
            axis=1,
        )
        in_maps.append({"body": body, "tails": np.ascontiguousarray(tails)})

    if _trace:
        res = run_bass_kernel_spmd(nc, in_maps, list(range(N_CORES)), trace=True)
    else:
        res = run_bass_kernel_spmd(nc, in_maps, list(range(N_CORES)))

    out = np.empty((2, BATCH, DIM, 1), dtype=np.float32)
    for i in range(N_CORES):
        ob = res.results[i]["out_body"]            # (2, B_PER, SPLIT)
        ot = res.results[i]["out_tail"]            # (NPART, 2*HK)
        sl = slice(i * B_PER, (i + 1) * B_PER)
        out[0, sl, :SPLIT, 0] = ob[0]
        out[1, sl, :SPLIT, 0] = ob[1]
        out[0, sl, SPLIT:, 0] = ot[:, :HK].reshape(B_PER, TAIL)
        out[1, sl, SPLIT:, 0] = ot[:, HK:].reshape(B_PER, TAIL)
    if _trace:
        kernel.last_results = res
    return out



# revision 4
# speedup vs baseline: 1.0858x; 1.0858x over previous
"""CP-gate layer kernel for Trainium2 (8 NeuronCores, batch-parallel).

The reference materializes the dense 2^n x 2^n CP gate, but that matrix is
diagonal: diag entry is e^{-i*phase} on basis states where both the control
(bit 11, MSB) and target (bit 10) bits are 1, else 1.  With MSB-first
ordering those states are exactly the contiguous index range [3072, 4096).
So U @ psi is: identity on k < 3072, and a fixed complex rotation of the
tail quarter.  The batch of 64 state vectors is sharded across 8 cores
(8 states/core): each core DMA-copies the untouched 3/4 DRAM->DRAM and
rotates its tail quarter on the vector engine.

The kernel is raw manually-synced bacc (no TileContext): the whole job is
one load->rotate->store latency chain plus one independent body copy, so
Tile's scheduling/barrier machinery only adds overhead.  Host packs the
tail as a contiguous (128, 128) tile ([re | im] along the free dim) so one
DMA feeds the rotate and DVE ops run at full width.

Critical-path choices (vs the straightforward version):
  - The tail load is the first SP instruction (before SP's start-barrier
    Drain), so its HWDGE gen + DGE->DMA pipeline starts at t~0.
  - The three DVE rotate ops chain on engine program order alone (the DVE
    pipeline-drain serializes same-engine ops in hardware); only the last
    op increments the `dve` sem that releases the store trigger.
  - The store is a PREPARE_ONLY kv_writeback triggered by gpsimd: the
    SWDGE descriptor gen (~1us) runs during the load window, and the
    triggered transfer skips the DGE->DMA handoff delay entirely.
  - The body copy carries no semaphore and nothing waits on it or the
    store at the end barrier: both DMAs are fired before the end barrier
    and complete before the queues drain; the end barrier only
    synchronizes engine halt.
  - The three unused const-AP preamble memsets (f32 1.0, bf16 1.0,
    u8 127) are removed so Pool reaches the store prep sooner.
"""

import numpy as np

N_CORES = 8
BATCH = 64
DIM = 4096
B_PER = BATCH // N_CORES          # 8 states per core
SPLIT = 3072                      # k >= SPLIT picks up the phase
TAIL = DIM - SPLIT                # 1024
NPART = 128                       # tail tile partitions: (b, km) = 8*16
HK = 64                           # tail tile cols per half: re 0:64, im 64:128
PHASE = np.pi / 4.0
C = float(np.cos(PHASE))          # cos == sin for pi/4

_cached_nc = None


def _build_nc():
    import concourse.bacc as bacc
    import concourse.bass as bass
    import concourse.mybir as mybir

    f32 = mybir.dt.float32
    i32 = mybir.dt.int32
    nc = bacc.Bacc("TRN2", target_bir_lowering=False, debug=False, num_devices=N_CORES)
    body = nc.declare_dram_parameter("body", [2, B_PER, SPLIT], f32, isOutput=False)
    tails = nc.declare_dram_parameter("tails", [NPART, 2 * HK], f32, isOutput=False)
    obody = nc.declare_dram_parameter("out_body", [2, B_PER, SPLIT], f32, isOutput=True)
    otail = nc.declare_dram_parameter("out_tail", [NPART, 2 * HK], f32, isOutput=True)

    with (
        nc.sbuf_tensor([NPART, 2 * HK], f32) as t,
        nc.sbuf_tensor([NPART, 2 * HK], f32) as s,
        nc.sbuf_tensor([NPART, 2 * HK], f32) as r,
        nc.Block() as block,
        nc.semaphore("ld") as ld,
        nc.semaphore("dve") as dve,
        nc.semaphore("cp") as cp,
        nc.semaphore("st") as st,
        nc.semaphore("prep") as prep,
    ):

        @block.sync
        def _(sp: bass.BassEngine):
            sp.dma_start(out=t[:], in_=tails[:]).then_inc(ld, 16)

        @block.gpsimd
        def _(g: bass.BassEngine):
            # The compiler requires sync info on every DGE DMA, so the body
            # copy carries a sem inc — but nothing waits on it.
            g.dma_start(out=obody[:, :, :], in_=body[:, :, :]).then_inc(cp, 16)
            # Zero ctx index: reuse the preamble's const-f32-0.0 [128,1] SBUF
            # tensor (all-zero bytes) bitcast to int32; Pool program order
            # (memset precedes the prep) makes the read safe.
            idx0 = nc.const_aps.aps[(f32, 0.0)].bitcast(i32)
            out4 = otail[:].rearrange("p (o n) -> p o n", o=1).unsqueeze(0)
            in4 = r[:].rearrange("p (a n) -> p a n", a=1).unsqueeze(2)
            g.kv_writeback(
                out_ap=out4, in_ap=in4, ctx_idxs_ap=idx0,
                prepare_only=True, sem=st, queue_num=0,
            ).then_inc(prep, 1)
            # Wait order matters: Bacc fuses one wait onto the trigger.
            # This order lands the critical dve wait ON the trigger ISA op
            # and leaves the early-satisfied prep wait standalone (~60 ns).
            g.wait_ge(dve, 1)
            g.wait_ge(prep, 1)
            g.trigger_dma(count=1, queue_num=0)

        @block.vector
        def _(v: bass.BassEngine):
            v.wait_ge(ld, 16)
            # s_im = fl(C*im); then out_re = fl(C*re)+s_im, out_im = s_im-fl(C*re)
            # via scalar_tensor_tensor — same rounding as the reference.
            # The three ops chain on DVE program order (no sems needed:
            # the engine's pipeline drain serializes same-engine ops).
            v.tensor_scalar_mul(s[:, HK : 2 * HK], t[:, HK : 2 * HK], C)
            v.scalar_tensor_tensor(
                out=r[:, 0:HK], in0=t[:, 0:HK], scalar=C, in1=s[:, HK : 2 * HK],
                op0=mybir.AluOpType.mult, op1=mybir.AluOpType.add,
            )
            v.scalar_tensor_tensor(
                out=r[:, HK : 2 * HK], in0=t[:, 0:HK], scalar=-C, in1=s[:, HK : 2 * HK],
                op0=mybir.AluOpType.mult, op1=mybir.AluOpType.add,
            ).then_inc(dve, 1)

    SP = mybir.EngineType.SP
    Pool = mybir.EngineType.Pool
    fn = nc.m.functions[0]
    main = fn.blocks[0]

    # Drop the three unused const-AP preamble memsets (f32 1.0, bf16 1.0,
    # u8 127) so Pool's engine is free for the body-copy SWDGE gen and the
    # store prep sooner.  The first memset (const-f32-0.0) stays: the store
    # prep reads it as its zero ctx index.
    memsets = [i for i in main.instructions if isinstance(i, mybir.InstMemset)]
    assert len(memsets) == 4, len(memsets)
    for i in memsets[1:]:
        main.instructions.remove(i)

    # Hoist the tail load to the very top of SP's stream (before its
    # start-barrier Drain): its ~2.4us DMA pipeline starts at t~0.  Safe:
    # sems are zeroed by the NRT preamble before any instruction runs, the
    # load waits on nothing, and its sem inc lands long after all engines
    # left the barrier.
    load_inst = None
    for b in fn.blocks:
        for i in list(b.instructions):
            if isinstance(i, mybir.InstDMACopy) and i.engine == SP:
                load_inst = i
                b.instructions.remove(i)
                break
        if load_inst is not None:
            break
    assert load_inst is not None
    for n, i in enumerate(main.instructions):
        if isinstance(i, mybir.InstDrain) and i.engine == SP:
            main.instructions.insert(n, load_inst)
            break
    else:
        raise AssertionError("SP start-barrier Drain not found")

    # Hoist the body copy's dispatch into Pool's barrier window (after its
    # Drain, before its gather EventSemaphore): the ~1us SWDGE descriptor
    # gen runs during the barrier.  Same safety argument as the load hoist.
    copy_inst = None
    for b in fn.blocks:
        for i in list(b.instructions):
            if isinstance(i, mybir.InstDMACopy) and i.engine == Pool:
                copy_inst = i
                b.instructions.remove(i)
                break
        if copy_inst is not None:
            break
    assert copy_inst is not None
    for n, i in enumerate(main.instructions):
        if isinstance(i, mybir.InstEventSemaphore) and i.engine == Pool:
            main.instructions.insert(n, copy_inst)
            break
    else:
        raise AssertionError("Pool barrier EventSemaphore not found")

    nc.finalize()
    return nc


def _get_nc():
    global _cached_nc
    if _cached_nc is None:
        _cached_nc = _build_nc()
    return _cached_nc


def kernel(psi_re=None, psi_im=None, U_re=None, U_im=None, _trace=False, **_ignored):
    from concourse.bass_utils import run_bass_kernel_spmd

    psi_re = np.asarray(psi_re, dtype=np.float32).reshape(BATCH, DIM)
    psi_im = np.asarray(psi_im, dtype=np.float32).reshape(BATCH, DIM)

    nc = _get_nc()
    in_maps = []
    for i in range(N_CORES):
        re = psi_re[i * B_PER : (i + 1) * B_PER]
        im = psi_im[i * B_PER : (i + 1) * B_PER]
        body = np.ascontiguousarray(np.stack([re[:, :SPLIT], im[:, :SPLIT]]))
        tails = np.concatenate(
            [re[:, SPLIT:].reshape(NPART, HK), im[:, SPLIT:].reshape(NPART, HK)],
            axis=1,
        )
        in_maps.append({"body": body, "tails": np.ascontiguousarray(tails)})

    if _trace:
        res = run_bass_kernel_spmd(nc, in_maps, list(range(N_CORES)), trace=True)
    else:
        res = run_bass_kernel_spmd(nc, in_maps, list(range(N_CORES)))

    out = np.empty((2, BATCH, DIM, 1), dtype=np.float32)
    for i in range(N_CORES):
        ob = res.results[i]["out_body"]            # (2, B_PER, SPLIT)
        ot = res.results[i]["out_tail"]            # (NPART, 2*HK)
        sl = slice(i * B_PER, (i + 1) * B_PER)
        out[0, sl, :SPLIT, 0] = ob[0]
        out[1, sl, :SPLIT, 0] = ob[1]
        out[0, sl, SPLIT:, 0] = ot[:, :HK].reshape(B_PER, TAIL)
        out[1, sl, SPLIT:, 0] = ot[:, HK:].reshape(B_PER, TAIL)
    if _trace:
        kernel.last_results = res
    return out
